# revision 13
# baseline (speedup 1.0000x reference)
"""Grouped matmul (MoE routing) kernel for Trainium2, 8 NeuronCores.

Problem: y[t] = x[t] @ weight[e].T for tokens t in [starts[e], offs[e]),
with x [4096, 2048] f32, weight [8, 1024, 2048] f32, offs [8] int32
(cumulative group ends). Output [4096, 1024] f32; tokens >= offs[-1] -> 0.

Strategy: expert-parallel. Routing is done host-side (offs is a host
numpy array): core e receives its expert's token slice, transposed and
zero-padded to P rows (x_e^T [K, P]), plus its expert's transposed
weight (w_e^T [K, N]). Each core runs a dense [P,K]x[K,N] matmul; the
host scatters per-core outputs back into the full [T, N] result.

Matmul dtype modes (GMM_MODE env): fp32 (exact, 4 cyc/row), fp32r
(1 cyc/row at N=512), bf16 (1 cyc/row, half DMA), bf16x3 (hi/lo split,
near-fp32 accuracy, 3x bf16 compute).
"""

import math
import os
import sys

for _p in ("/opt/pypackages", "/opt/trn_rl_repo"):
    if _p not in sys.path:
        sys.path.insert(0, _p)

import numpy as np

E, K, N, T = 8, 2048, 1024, 4096
NCORES = 8
KT = 128  # contraction tile (PE partition dim)
NT = 512  # psum free-dim chunk (one PSUM bank of f32)
MB = 512  # m-block rows kept resident in SBUF at once

MODE = os.environ.get("GMM_MODE", "fp32r")
TRACE = bool(int(os.environ.get("GMM_TRACE", "0")))

_nc_cache = {}
last_result = None  # BassKernelResults of the most recent run (for test.py)


def _dtypes(mode):
    from concourse import mybir

    if mode == "fp32":
        return mybir.dt.float32, np.float32
    if mode == "fp32r":
        return mybir.dt.float32r, np.float32
    import ml_dtypes

    return mybir.dt.bfloat16, np.dtype(ml_dtypes.bfloat16)


def _build_v4(P, mode):
    """v3 + host-pre-tiled inputs ([128, K/128, cols] layout -> 8KB DMA
    runs), KS=4, all stores on SWDGE. Single-tensor modes only."""
    import concourse.tile as tile
    from concourse import bacc, mybir

    f32 = mybir.dt.float32
    mmdt, _ = _dtypes(mode)

    KS = 4
    n_slab = K // (128 * KS)  # 4
    n_k = K // 128
    n_m = P // 128
    assert n_m <= 8
    n_half = N // 2

    nc = bacc.Bacc(
        "TRN2", target_bir_lowering=False, debug=False, num_devices=NCORES
    )

    w0 = nc.dram_tensor("wT0", [128, n_k, n_half], mmdt, kind="ExternalInput").ap()
    w1 = nc.dram_tensor("wT1", [128, n_k, n_half], mmdt, kind="ExternalInput").ap()
    xt = nc.dram_tensor("xTt", [128, n_k, P], mmdt, kind="ExternalInput").ap()
    y = nc.dram_tensor("y", [P, N], f32, kind="ExternalOutput").ap()

    WKS = 2  # w slab k-subtiles (finer pacing near stream end)
    n_wslab = K // (128 * WKS)

    with tile.TileContext(nc) as tc:
        with (
            tc.tile_pool(name="w0", bufs=n_wslab) as w0pool,
            tc.tile_pool(name="w1", bufs=n_wslab) as w1pool,
            tc.tile_pool(name="x", bufs=n_slab) as xpool,
            tc.tile_pool(name="ps", bufs=8, space="PSUM") as pspool,
            tc.tile_pool(name="o", bufs=8) as opool,
        ):
            # ring A (sync): w0_s, w1_s interleaved in k order so both
            # n-halves' slab k arrives ~together; ring B (scalar): x.
            w0_slabs, w1_slabs, x_slabs = [], [], []
            for s in range(n_wslab):
                ks = slice(s * WKS, (s + 1) * WKS)
                t = w0pool.tile([128, WKS, n_half], mmdt, tag="w0", name=f"w0s{s}")
                nc.sync.dma_start(t[:], w0[:, ks, :])
                w0_slabs.append(t)
                t = w1pool.tile([128, WKS, n_half], mmdt, tag="w1", name=f"w1s{s}")
                nc.sync.dma_start(t[:], w1[:, ks, :])
                w1_slabs.append(t)
            for s in range(n_slab):
                ks = slice(s * KS, (s + 1) * KS)
                t = xpool.tile([128, KS, P], mmdt, tag="x", name=f"xs{s}")
                nc.scalar.dma_start(t[:], xt[:, ks, :])
                x_slabs.append(t)

            ps_tiles = [
                pspool.tile([128, n_half], f32, tag="ps", name=f"ps{h}_{i}")
                for h in range(2)
                for i in range(n_m)
            ]
            w_halves = [w0_slabs, w1_slabs]
            for k in range(n_k):
                ws, wj = divmod(k, WKS)
                xs, xj = divmod(k, KS)
                for h in range(2):
                    for mi in range(n_m):
                        nc.tensor.matmul(
                            ps_tiles[h * n_m + mi][:, :],
                            x_slabs[xs][:, xj, mi * 128 : (mi + 1) * 128],
                            w_halves[h][ws][:, wj, :],
                            start=(k == 0),
                            stop=(k == n_k - 1),
                        )
            for h in range(2):
                for mi in range(n_m):
                    ot = opool.tile(
                        [128, n_half], f32, tag="o", name=f"o{h}_{mi}"
                    )
                    nc.vector.tensor_copy(ot[:], ps_tiles[h * n_m + mi][:])
                    eng = nc.sync if (h * n_m + mi) % 2 == 0 else nc.scalar
                    eng.dma_start(
                        y[
                            mi * 128 : (mi + 1) * 128,
                            h * n_half : (h + 1) * n_half,
                        ],
                        ot[:],
                    )

    nc.compile()
    return nc


def _build_v3(P, mode):
    """k-outer over all PSUM banks, n-half waves for early output overlap,
    slab DMAs balanced across both HWDGE rings. P <= 1024."""
    import concourse.tile as tile
    from concourse import bacc, mybir

    f32 = mybir.dt.float32
    mmdt, _ = _dtypes(mode)
    two = mode == "bf16x3"

    KS = 2  # k-subtiles per DMA slab
    n_slab = K // (128 * KS)  # 8
    n_k = K // 128  # 16
    n_m = P // 128
    assert n_m <= 8
    n_half = N // 2  # 512: one psum bank per (m, half)

    nc = bacc.Bacc(
        "TRN2", target_bir_lowering=False, debug=False, num_devices=NCORES
    )

    def din(name, shape):
        return nc.dram_tensor(name, shape, mmdt, kind="ExternalInput").ap()

    y = nc.dram_tensor("y", [P, N], f32, kind="ExternalOutput").ap()
    if two:
        x_ins = [din("x_hi", [K, P]), din("x_lo", [K, P])]
        w_ins = [din("w_hi", [K, N]), din("w_lo", [K, N])]
    else:
        x_ins = [din("xT", [K, P])]
        w_ins = [din("wT", [K, N])]

    x_views = [a.rearrange("(po pi) f -> pi po f", pi=128) for a in x_ins]
    w_views = [a.rearrange("(po pi) f -> pi po f", pi=128) for a in w_ins]
    nw = len(w_ins)
    nx = len(x_ins)

    with tile.TileContext(nc) as tc:
        with (
            tc.tile_pool(name="w0", bufs=n_slab * nw) as w0pool,
            tc.tile_pool(name="w1", bufs=n_slab * nw) as w1pool,
            tc.tile_pool(name="x", bufs=n_slab * nx) as xpool,
            tc.tile_pool(name="ps", bufs=8, space="PSUM") as pspool,
            tc.tile_pool(name="o", bufs=8) as opool,
        ):
            # ring A (sync): w n-half 0 slabs; ring B (scalar): x slabs.
            # Then w n-half 1 slabs split across both rings.
            w0_slabs, w1_slabs, x_slabs = [], [], []
            for s in range(n_slab):
                ks = slice(s * KS, (s + 1) * KS)
                row = []
                for wv in w_views:
                    t = w0pool.tile([128, KS, n_half], mmdt, tag="w0")
                    nc.sync.dma_start(t[:], wv[:, ks, 0:n_half])
                    row.append(t)
                w0_slabs.append(row)
                row = []
                for xv in x_views:
                    t = xpool.tile([128, KS, P], mmdt, tag="x")
                    nc.scalar.dma_start(t[:], xv[:, ks, :])
                    row.append(t)
                x_slabs.append(row)
            for s in range(n_slab):
                ks = slice(s * KS, (s + 1) * KS)
                eng = nc.sync if s % 2 == 0 else nc.scalar
                row = []
                for wv in w_views:
                    t = w1pool.tile([128, KS, n_half], mmdt, tag="w1")
                    eng.dma_start(t[:], wv[:, ks, n_half:N])
                    row.append(t)
                w1_slabs.append(row)

            prods = [(0, 0)] if not two else [(0, 0), (1, 0), (0, 1)]
            n_acc = n_k * len(prods)

            def wave(w_slabs, ncol0, store_engines):
                ps_tiles = [
                    pspool.tile([128, n_half], f32, tag="ps", name=f"ps{i}")
                    for i in range(n_m)
                ]
                for k in range(n_k):
                    s, j = divmod(k, KS)
                    for mi in range(n_m):
                        i_acc0 = k * len(prods)
                        for pi, (xi, wi) in enumerate(prods):
                            nc.tensor.matmul(
                                ps_tiles[mi][:, :],
                                x_slabs[s][xi][:, j, mi * 128 : (mi + 1) * 128],
                                w_slabs[s][wi][:, j, :],
                                start=(i_acc0 + pi == 0),
                                stop=(i_acc0 + pi == n_acc - 1),
                            )
                for mi in range(n_m):
                    ot = opool.tile([128, n_half], f32, tag="o")
                    nc.vector.tensor_copy(ot[:], ps_tiles[mi][:])
                    eng = store_engines[mi % len(store_engines)]
                    eng.dma_start(
                        y[mi * 128 : (mi + 1) * 128, ncol0 : ncol0 + n_half], ot[:]
                    )

            # n-half 0 completes mid-stream; store via SWDGE to keep HWDGE
            # rings on input. n-half 1 stores at the end on the idle rings.
            wave(w0_slabs, 0, [nc.gpsimd])
            wave(w1_slabs, n_half, [nc.sync, nc.scalar])

    nc.compile()
    return nc


def _build_v2(P, mode):
    """Lean hand-rolled kernel: slab DMAs on both HWDGE rings, k-inner
    accumulation, outputs via SWDGE. P must be <= 1024."""
    import concourse.tile as tile
    from concourse import bacc, mybir

    f32 = mybir.dt.float32
    mmdt, _ = _dtypes(mode)
    two = mode == "bf16x3"

    KS = 4  # k-subtiles per DMA slab
    n_slab = K // (128 * KS)
    n_k = K // 128
    n_m = P // 128
    n_n = N // NT

    nc = bacc.Bacc(
        "TRN2", target_bir_lowering=False, debug=False, num_devices=NCORES
    )

    def din(name, shape):
        return nc.dram_tensor(name, shape, mmdt, kind="ExternalInput").ap()

    y = nc.dram_tensor("y", [P, N], f32, kind="ExternalOutput").ap()
    if two:
        x_ins = [din("x_hi", [K, P]), din("x_lo", [K, P])]
        w_ins = [din("w_hi", [K, N]), din("w_lo", [K, N])]
    else:
        x_ins = [din("xT", [K, P])]
        w_ins = [din("wT", [K, N])]

    x_views = [a.rearrange("(po pi) f -> pi po f", pi=128) for a in x_ins]
    w_views = [a.rearrange("(po pi) f -> pi po f", pi=128) for a in w_ins]

    with tile.TileContext(nc) as tc:
        with (
            tc.tile_pool(name="w", bufs=n_slab * len(w_ins)) as wpool,
            tc.tile_pool(name="x", bufs=n_slab * len(x_ins)) as xpool,
            tc.tile_pool(name="ps", bufs=4, space="PSUM") as pspool,
            tc.tile_pool(name="o", bufs=4) as opool,
        ):
            w_slabs, x_slabs = [], []
            for s in range(n_slab):
                ks = slice(s * KS, (s + 1) * KS)
                wrow, xrow = [], []
                for wi, wv in enumerate(w_views):
                    t = wpool.tile([128, KS, N], mmdt, tag="w")
                    nc.sync.dma_start(t[:], wv[:, ks, :])
                    wrow.append(t)
                for xi, xv in enumerate(x_views):
                    t = xpool.tile([128, KS, P], mmdt, tag="x")
                    nc.scalar.dma_start(t[:], xv[:, ks, :])
                    xrow.append(t)
                w_slabs.append(wrow)
                x_slabs.append(xrow)

            prods = [(0, 0)] if not two else [(0, 0), (1, 0), (0, 1)]
            n_acc = n_k * len(prods)
            for mi in range(n_m):
                ms = slice(mi * 128, (mi + 1) * 128)
                for ni in range(n_n):
                    nsl = slice(ni * NT, (ni + 1) * NT)
                    ps = pspool.tile([128, NT], f32, tag="ps")
                    i_acc = 0
                    for k in range(n_k):
                        s, j = divmod(k, KS)
                        for xi, wi in prods:
                            nc.tensor.matmul(
                                ps[:, :],
                                x_slabs[s][xi][:, j, ms],
                                w_slabs[s][wi][:, j, nsl],
                                start=(i_acc == 0),
                                stop=(i_acc == n_acc - 1),
                            )
                            i_acc += 1
                    ot = opool.tile([128, NT], f32, tag="o")
                    nc.vector.tensor_copy(ot[:], ps[:])
                    nc.gpsimd.dma_start(y[ms, nsl], ot[:])

    nc.compile()
    return nc


def _build(P, mode):
    import concourse.tile as tile
    from concourse import bacc, mybir
    from concourse.kernels.tile_matmul import matmul_tile_kernel

    f32 = mybir.dt.float32
    mmdt, _ = _dtypes(mode)
    two = mode == "bf16x3"  # hi/lo split inputs

    nc = bacc.Bacc(
        "TRN2", target_bir_lowering=False, debug=False, num_devices=NCORES
    )

    def din(name, shape):
        return nc.dram_tensor(name, shape, mmdt, kind="ExternalInput").ap()

    y = nc.dram_tensor("y", [P, N], f32, kind="ExternalOutput").ap()
    if two:
        x_hi, x_lo = din("x_hi", [K, P]), din("x_lo", [K, P])
        w_hi, w_lo = din("w_hi", [K, N]), din("w_lo", [K, N])
    else:
        xT, wT = din("xT", [K, P]), din("wT", [K, N])

    with tile.TileContext(nc) as tc:
        if two:
            # y = xhi.T@whi + xlo.T@whi + xhi.T@wlo, accumulated via DMA
            matmul_tile_kernel(tc, x_hi, w_hi, y)
            matmul_tile_kernel(tc, x_lo, w_hi, y, mxn_accum_op=mybir.AluOpType.add)
            matmul_tile_kernel(tc, x_hi, w_lo, y, mxn_accum_op=mybir.AluOpType.add)
        else:
            matmul_tile_kernel(tc, xT, wT, y)

    nc.compile()
    return nc


KERNEL_V = os.environ.get("GMM_KERNEL", "v4")


def _use_v4(P, mode):
    return KERNEL_V == "v4" and P <= 1024 and mode != "bf16x3"


def _get_nc(P, mode):
    key = (P, mode, KERNEL_V)
    if key not in _nc_cache:
        if _use_v4(P, mode):
            _nc_cache[key] = _build_v4(P, mode)
        elif KERNEL_V in ("v3", "v4") and P <= 1024:
            _nc_cache[key] = _build_v3(P, mode)
        elif KERNEL_V == "v2" and P <= 1024:
            _nc_cache[key] = _build_v2(P, mode)
        else:
            _nc_cache[key] = _build(P, mode)
    return _nc_cache[key]


def _split_hi_lo(a, np_bf16):
    hi = a.astype(np_bf16)
    lo = (a - hi.astype(np.float32)).astype(np_bf16)
    return hi, lo


def kernel(x, weight, offs):
    global last_result
    from concourse.bass_utils import run_bass_kernel_spmd

    x = np.ascontiguousarray(x, dtype=np.float32)
    weight = np.ascontiguousarray(weight, dtype=np.float32)
    offs = np.asarray(offs, dtype=np.int64)

    starts = np.zeros(E, dtype=np.int64)
    starts[1:] = offs[:-1]
    starts = np.clip(starts, 0, T)
    ends = np.clip(offs, 0, T)
    sizes = np.maximum(ends - starts, 0)

    P = max(128, int(math.ceil(max(int(sizes.max()), 1) / 128.0)) * 128)
    mode = MODE
    _, np_in = _dtypes(mode)

    nc = _get_nc(P, mode)

    in_maps = []
    for e in range(E):
        xe = x[starts[e] : starts[e] + sizes[e]]
        xT = np.zeros((K, P), dtype=np.float32)
        xT[:, : sizes[e]] = xe.T
        wT = np.ascontiguousarray(weight[e].T)  # [K, N]
        if _use_v4(P, mode):
            # pre-tiled [pi, po, cols] layout, k = po*128 + pi
            def tile3(a):
                return np.ascontiguousarray(
                    a.reshape(K // 128, 128, a.shape[1]).transpose(1, 0, 2)
                ).astype(np_in)

            in_maps.append(
                {
                    "wT0": tile3(wT[:, : N // 2]),
                    "wT1": tile3(wT[:, N // 2 :]),
                    "xTt": tile3(xT),
                }
            )
            continue
        if mode == "bf16x3":
            import ml_dtypes

            bf = np.dtype(ml_dtypes.bfloat16)
            x_hi, x_lo = _split_hi_lo(xT, bf)
            w_hi, w_lo = _split_hi_lo(wT, bf)
            in_maps.append(
                {"x_hi": x_hi, "x_lo": x_lo, "w_hi": w_hi, "w_lo": w_lo}
            )
        elif mode == "bf16":
            in_maps.append({"xT": xT.astype(np_in), "wT": wT.astype(np_in)})
        else:
            in_maps.append({"xT": xT, "wT": wT})

    res = run_bass_kernel_spmd(
        nc, in_maps, list(range(NCORES)), trace=TRACE
    )
    last_result = res

    out = np.zeros((T, N), dtype=np.float32)
    for e in range(E):
        if sizes[e]:
            out[starts[e] : ends[e]] = res.results[e]["y"][: sizes[e]]
    return out


# revision 14
# speedup vs baseline: 1.1217x; 1.1217x over previous
"""Grouped matmul (MoE routing) kernel for Trainium2, 8 NeuronCores.

Problem: y[t] = x[t] @ weight[e].T for tokens t in [starts[e], offs[e]),
with x [4096, 2048] f32, weight [8, 1024, 2048] f32, offs [8] int32
(cumulative group ends). Output [4096, 1024] f32; tokens >= offs[-1] -> 0.

Strategy: expert-parallel. Routing is done host-side (offs is a host
numpy array): core e receives its expert's token slice, transposed and
zero-padded to P rows (x_e^T [K, P]), plus its expert's transposed
weight (w_e^T [K, N]). Each core runs a dense [P,K]x[K,N] matmul; the
host scatters per-core outputs back into the full [T, N] result.

Matmul dtype modes (GMM_MODE env): fp32 (exact, 4 cyc/row), fp32r
(1 cyc/row at N=512), bf16 (1 cyc/row, half DMA), bf16x3 (hi/lo split,
near-fp32 accuracy, 3x bf16 compute).
"""

import math
import os
import sys

for _p in ("/opt/pypackages", "/opt/trn_rl_repo"):
    if _p not in sys.path:
        sys.path.insert(0, _p)

import numpy as np

E, K, N, T = 8, 2048, 1024, 4096
NCORES = 8
KT = 128  # contraction tile (PE partition dim)
NT = 512  # psum free-dim chunk (one PSUM bank of f32)
MB = 512  # m-block rows kept resident in SBUF at once

MODE = os.environ.get("GMM_MODE", "fp32r")
TRACE = bool(int(os.environ.get("GMM_TRACE", "0")))

_nc_cache = {}
last_result = None  # BassKernelResults of the most recent run (for test.py)


def _dtypes(mode):
    from concourse import mybir

    if mode == "fp32":
        return mybir.dt.float32, np.float32
    if mode == "fp32r":
        return mybir.dt.float32r, np.float32
    import ml_dtypes

    return mybir.dt.bfloat16, np.dtype(ml_dtypes.bfloat16)


def _build_v4(P, mode):
    """v3 + host-pre-tiled inputs ([128, K/128, cols] layout -> 8KB DMA
    runs), KS=4, all stores on SWDGE. Single-tensor modes only."""
    import concourse.tile as tile
    from concourse import bacc, mybir

    f32 = mybir.dt.float32
    mmdt, _ = _dtypes(mode)

    KS = 4
    n_slab = K // (128 * KS)  # 4
    n_k = K // 128
    n_m = P // 128
    assert n_m <= 8
    n_half = N // 2

    nc = bacc.Bacc(
        "TRN2", target_bir_lowering=False, debug=False, num_devices=NCORES
    )

    w0 = nc.dram_tensor("wT0", [128, n_k, n_half], mmdt, kind="ExternalInput").ap()
    w1 = nc.dram_tensor("wT1", [128, n_k, n_half], mmdt, kind="ExternalInput").ap()
    xt = nc.dram_tensor("xTt", [128, n_k, P], mmdt, kind="ExternalInput").ap()
    y = nc.dram_tensor("y", [P, N], f32, kind="ExternalOutput").ap()

    WKS = 2  # w slab k-subtiles (finer pacing near stream end)
    n_wslab = K // (128 * WKS)

    with tile.TileContext(nc) as tc:
        with (
            tc.tile_pool(name="w0", bufs=n_wslab) as w0pool,
            tc.tile_pool(name="w1", bufs=n_wslab) as w1pool,
            tc.tile_pool(name="x", bufs=n_slab) as xpool,
            tc.tile_pool(name="ps", bufs=8, space="PSUM") as pspool,
            tc.tile_pool(name="o", bufs=8) as opool,
        ):
            # Balanced rings (~6.3MB each), w slabs arriving in k order:
            # ring A (sync): w0/w1 slabs k 0..11; ring B (scalar): x, then
            # w0/w1 slabs k 12..15 (the final MM chain's inputs).
            w0_slabs = [None] * n_wslab
            w1_slabs = [None] * n_wslab
            x_slabs = []
            for s in range(n_wslab - 2):
                ks = slice(s * WKS, (s + 1) * WKS)
                t = w0pool.tile([128, WKS, n_half], mmdt, tag="w0", name=f"w0s{s}")
                nc.sync.dma_start(t[:], w0[:, ks, :])
                w0_slabs[s] = t
                t = w1pool.tile([128, WKS, n_half], mmdt, tag="w1", name=f"w1s{s}")
                nc.sync.dma_start(t[:], w1[:, ks, :])
                w1_slabs[s] = t
            for s in range(n_slab):
                ks = slice(s * KS, (s + 1) * KS)
                t = xpool.tile([128, KS, P], mmdt, tag="x", name=f"xs{s}")
                nc.scalar.dma_start(t[:], xt[:, ks, :])
                x_slabs.append(t)
            for s in range(n_wslab - 2, n_wslab):
                ks = slice(s * WKS, (s + 1) * WKS)
                t = w0pool.tile([128, WKS, n_half], mmdt, tag="w0", name=f"w0s{s}")
                nc.scalar.dma_start(t[:], w0[:, ks, :])
                w0_slabs[s] = t
                t = w1pool.tile([128, WKS, n_half], mmdt, tag="w1", name=f"w1s{s}")
                nc.scalar.dma_start(t[:], w1[:, ks, :])
                w1_slabs[s] = t

            ps_tiles = [
                pspool.tile([128, n_half], f32, tag="ps", name=f"ps{h}_{i}")
                for h in range(2)
                for i in range(n_m)
            ]
            w_halves = [w0_slabs, w1_slabs]
            for k in range(n_k):
                ws, wj = divmod(k, WKS)
                xs, xj = divmod(k, KS)
                for h in range(2):
                    for mi in range(n_m):
                        nc.tensor.matmul(
                            ps_tiles[h * n_m + mi][:, :],
                            x_slabs[xs][:, xj, mi * 128 : (mi + 1) * 128],
                            w_halves[h][ws][:, wj, :],
                            start=(k == 0),
                            stop=(k == n_k - 1),
                        )
            for h in range(2):
                for mi in range(n_m):
                    ot = opool.tile(
                        [128, n_half], f32, tag="o", name=f"o{h}_{mi}"
                    )
                    nc.vector.tensor_copy(ot[:], ps_tiles[h * n_m + mi][:])
                    eng = nc.sync if (h * n_m + mi) % 2 == 0 else nc.scalar
                    eng.dma_start(
                        y[
                            mi * 128 : (mi + 1) * 128,
                            h * n_half : (h + 1) * n_half,
                        ],
                        ot[:],
                    )

    nc.compile()
    return nc


def _build_v3(P, mode):
    """k-outer over all PSUM banks, n-half waves for early output overlap,
    slab DMAs balanced across both HWDGE rings. P <= 1024."""
    import concourse.tile as tile
    from concourse import bacc, mybir

    f32 = mybir.dt.float32
    mmdt, _ = _dtypes(mode)
    two = mode == "bf16x3"

    KS = 2  # k-subtiles per DMA slab
    n_slab = K // (128 * KS)  # 8
    n_k = K // 128  # 16
    n_m = P // 128
    assert n_m <= 8
    n_half = N // 2  # 512: one psum bank per (m, half)

    nc = bacc.Bacc(
        "TRN2", target_bir_lowering=False, debug=False, num_devices=NCORES
    )

    def din(name, shape):
        return nc.dram_tensor(name, shape, mmdt, kind="ExternalInput").ap()

    y = nc.dram_tensor("y", [P, N], f32, kind="ExternalOutput").ap()
    if two:
        x_ins = [din("x_hi", [K, P]), din("x_lo", [K, P])]
        w_ins = [din("w_hi", [K, N]), din("w_lo", [K, N])]
    else:
        x_ins = [din("xT", [K, P])]
        w_ins = [din("wT", [K, N])]

    x_views = [a.rearrange("(po pi) f -> pi po f", pi=128) for a in x_ins]
    w_views = [a.rearrange("(po pi) f -> pi po f", pi=128) for a in w_ins]
    nw = len(w_ins)
    nx = len(x_ins)

    with tile.TileContext(nc) as tc:
        with (
            tc.tile_pool(name="w0", bufs=n_slab * nw) as w0pool,
            tc.tile_pool(name="w1", bufs=n_slab * nw) as w1pool,
            tc.tile_pool(name="x", bufs=n_slab * nx) as xpool,
            tc.tile_pool(name="ps", bufs=8, space="PSUM") as pspool,
            tc.tile_pool(name="o", bufs=8) as opool,
        ):
            # ring A (sync): w n-half 0 slabs; ring B (scalar): x slabs.
            # Then w n-half 1 slabs split across both rings.
            w0_slabs, w1_slabs, x_slabs = [], [], []
            for s in range(n_slab):
                ks = slice(s * KS, (s + 1) * KS)
                row = []
                for wv in w_views:
                    t = w0pool.tile([128, KS, n_half], mmdt, tag="w0")
                    nc.sync.dma_start(t[:], wv[:, ks, 0:n_half])
                    row.append(t)
                w0_slabs.append(row)
                row = []
                for xv in x_views:
                    t = xpool.tile([128, KS, P], mmdt, tag="x")
                    nc.scalar.dma_start(t[:], xv[:, ks, :])
                    row.append(t)
                x_slabs.append(row)
            for s in range(n_slab):
                ks = slice(s * KS, (s + 1) * KS)
                eng = nc.sync if s % 2 == 0 else nc.scalar
                row = []
                for wv in w_views:
                    t = w1pool.tile([128, KS, n_half], mmdt, tag="w1")
                    eng.dma_start(t[:], wv[:, ks, n_half:N])
                    row.append(t)
                w1_slabs.append(row)

            prods = [(0, 0)] if not two else [(0, 0), (1, 0), (0, 1)]
            n_acc = n_k * len(prods)

            def wave(w_slabs, ncol0, store_engines):
                ps_tiles = [
                    pspool.tile([128, n_half], f32, tag="ps", name=f"ps{i}")
                    for i in range(n_m)
                ]
                for k in range(n_k):
                    s, j = divmod(k, KS)
                    for mi in range(n_m):
                        i_acc0 = k * len(prods)
                        for pi, (xi, wi) in enumerate(prods):
                            nc.tensor.matmul(
                                ps_tiles[mi][:, :],
                                x_slabs[s][xi][:, j, mi * 128 : (mi + 1) * 128],
                                w_slabs[s][wi][:, j, :],
                                start=(i_acc0 + pi == 0),
                                stop=(i_acc0 + pi == n_acc - 1),
                            )
                for mi in range(n_m):
                    ot = opool.tile([128, n_half], f32, tag="o")
                    nc.vector.tensor_copy(ot[:], ps_tiles[mi][:])
                    eng = store_engines[mi % len(store_engines)]
                    eng.dma_start(
                        y[mi * 128 : (mi + 1) * 128, ncol0 : ncol0 + n_half], ot[:]
                    )

            # n-half 0 completes mid-stream; store via SWDGE to keep HWDGE
            # rings on input. n-half 1 stores at the end on the idle rings.
            wave(w0_slabs, 0, [nc.gpsimd])
            wave(w1_slabs, n_half, [nc.sync, nc.scalar])

    nc.compile()
    return nc


def _build_v2(P, mode):
    """Lean hand-rolled kernel: slab DMAs on both HWDGE rings, k-inner
    accumulation, outputs via SWDGE. P must be <= 1024."""
    import concourse.tile as tile
    from concourse import bacc, mybir

    f32 = mybir.dt.float32
    mmdt, _ = _dtypes(mode)
    two = mode == "bf16x3"

    KS = 4  # k-subtiles per DMA slab
    n_slab = K // (128 * KS)
    n_k = K // 128
    n_m = P // 128
    n_n = N // NT

    nc = bacc.Bacc(
        "TRN2", target_bir_lowering=False, debug=False, num_devices=NCORES
    )

    def din(name, shape):
        return nc.dram_tensor(name, shape, mmdt, kind="ExternalInput").ap()

    y = nc.dram_tensor("y", [P, N], f32, kind="ExternalOutput").ap()
    if two:
        x_ins = [din("x_hi", [K, P]), din("x_lo", [K, P])]
        w_ins = [din("w_hi", [K, N]), din("w_lo", [K, N])]
    else:
        x_ins = [din("xT", [K, P])]
        w_ins = [din("wT", [K, N])]

    x_views = [a.rearrange("(po pi) f -> pi po f", pi=128) for a in x_ins]
    w_views = [a.rearrange("(po pi) f -> pi po f", pi=128) for a in w_ins]

    with tile.TileContext(nc) as tc:
        with (
            tc.tile_pool(name="w", bufs=n_slab * len(w_ins)) as wpool,
            tc.tile_pool(name="x", bufs=n_slab * len(x_ins)) as xpool,
            tc.tile_pool(name="ps", bufs=4, space="PSUM") as pspool,
            tc.tile_pool(name="o", bufs=4) as opool,
        ):
            w_slabs, x_slabs = [], []
            for s in range(n_slab):
                ks = slice(s * KS, (s + 1) * KS)
                wrow, xrow = [], []
                for wi, wv in enumerate(w_views):
                    t = wpool.tile([128, KS, N], mmdt, tag="w")
                    nc.sync.dma_start(t[:], wv[:, ks, :])
                    wrow.append(t)
                for xi, xv in enumerate(x_views):
                    t = xpool.tile([128, KS, P], mmdt, tag="x")
                    nc.scalar.dma_start(t[:], xv[:, ks, :])
                    xrow.append(t)
                w_slabs.append(wrow)
                x_slabs.append(xrow)

            prods = [(0, 0)] if not two else [(0, 0), (1, 0), (0, 1)]
            n_acc = n_k * len(prods)
            for mi in range(n_m):
                ms = slice(mi * 128, (mi + 1) * 128)
                for ni in range(n_n):
                    nsl = slice(ni * NT, (ni + 1) * NT)
                    ps = pspool.tile([128, NT], f32, tag="ps")
                    i_acc = 0
                    for k in range(n_k):
                        s, j = divmod(k, KS)
                        for xi, wi in prods:
                            nc.tensor.matmul(
                                ps[:, :],
                                x_slabs[s][xi][:, j, ms],
                                w_slabs[s][wi][:, j, nsl],
                                start=(i_acc == 0),
                                stop=(i_acc == n_acc - 1),
                            )
                            i_acc += 1
                    ot = opool.tile([128, NT], f32, tag="o")
                    nc.vector.tensor_copy(ot[:], ps[:])
                    nc.gpsimd.dma_start(y[ms, nsl], ot[:])

    nc.compile()
    return nc


def _build(P, mode):
    import concourse.tile as tile
    from concourse import bacc, mybir
    from concourse.kernels.tile_matmul import matmul_tile_kernel

    f32 = mybir.dt.float32
    mmdt, _ = _dtypes(mode)
    two = mode == "bf16x3"  # hi/lo split inputs

    nc = bacc.Bacc(
        "TRN2", target_bir_lowering=False, debug=False, num_devices=NCORES
    )

    def din(name, shape):
        return nc.dram_tensor(name, shape, mmdt, kind="ExternalInput").ap()

    y = nc.dram_tensor("y", [P, N], f32, kind="ExternalOutput").ap()
    if two:
        x_hi, x_lo = din("x_hi", [K, P]), din("x_lo", [K, P])
        w_hi, w_lo = din("w_hi", [K, N]), din("w_lo", [K, N])
    else:
        xT, wT = din("xT", [K, P]), din("wT", [K, N])

    with tile.TileContext(nc) as tc:
        if two:
            # y = xhi.T@whi + xlo.T@whi + xhi.T@wlo, accumulated via DMA
            matmul_tile_kernel(tc, x_hi, w_hi, y)
            matmul_tile_kernel(tc, x_lo, w_hi, y, mxn_accum_op=mybir.AluOpType.add)
            matmul_tile_kernel(tc, x_hi, w_lo, y, mxn_accum_op=mybir.AluOpType.add)
        else:
            matmul_tile_kernel(tc, xT, wT, y)

    nc.compile()
    return nc


KERNEL_V = os.environ.get("GMM_KERNEL", "v4")


def _use_v4(P, mode):
    return KERNEL_V == "v4" and P <= 1024 and mode != "bf16x3"


def _get_nc(P, mode):
    key = (P, mode, KERNEL_V)
    if key not in _nc_cache:
        if _use_v4(P, mode):
            _nc_cache[key] = _build_v4(P, mode)
        elif KERNEL_V in ("v3", "v4") and P <= 1024:
            _nc_cache[key] = _build_v3(P, mode)
        elif KERNEL_V == "v2" and P <= 1024:
            _nc_cache[key] = _build_v2(P, mode)
        else:
            _nc_cache[key] = _build(P, mode)
    return _nc_cache[key]


def _split_hi_lo(a, np_bf16):
    hi = a.astype(np_bf16)
    lo = (a - hi.astype(np.float32)).astype(np_bf16)
    return hi, lo


def kernel(x, weight, offs):
    global last_result
    from concourse.bass_utils import run_bass_kernel_spmd

    x = np.ascontiguousarray(x, dtype=np.float32)
    weight = np.ascontiguousarray(weight, dtype=np.float32)
    offs = np.asarray(offs, dtype=np.int64)

    starts = np.zeros(E, dtype=np.int64)
    starts[1:] = offs[:-1]
    starts = np.clip(starts, 0, T)
    ends = np.clip(offs, 0, T)
    sizes = np.maximum(ends - starts, 0)

    P = max(128, int(math.ceil(max(int(sizes.max()), 1) / 128.0)) * 128)
    mode = MODE
    _, np_in = _dtypes(mode)

    nc = _get_nc(P, mode)

    in_maps = []
    for e in range(E):
        xe = x[starts[e] : starts[e] + sizes[e]]
        xT = np.zeros((K, P), dtype=np.float32)
        xT[:, : sizes[e]] = xe.T
        wT = np.ascontiguousarray(weight[e].T)  # [K, N]
        if _use_v4(P, mode):
            # pre-tiled [pi, po, cols] layout, k = po*128 + pi
            def tile3(a):
                return np.ascontiguousarray(
                    a.reshape(K // 128, 128, a.shape[1]).transpose(1, 0, 2)
                ).astype(np_in)

            in_maps.append(
                {
                    "wT0": tile3(wT[:, : N // 2]),
                    "wT1": tile3(wT[:, N // 2 :]),
                    "xTt": tile3(xT),
                }
            )
            continue
        if mode == "bf16x3":
            import ml_dtypes

            bf = np.dtype(ml_dtypes.bfloat16)
            x_hi, x_lo = _split_hi_lo(xT, bf)
            w_hi, w_lo = _split_hi_lo(wT, bf)
            in_maps.append(
                {"x_hi": x_hi, "x_lo": x_lo, "w_hi": w_hi, "w_lo": w_lo}
            )
        elif mode == "bf16":
            in_maps.append({"xT": xT.astype(np_in), "wT": wT.astype(np_in)})
        else:
            in_maps.append({"xT": xT, "wT": wT})

    res = run_bass_kernel_spmd(
        nc, in_maps, list(range(NCORES)), trace=TRACE
    )
    last_result = res

    out = np.zeros((T, N), dtype=np.float32)
    for e in range(E):
        if sizes[e]:
            out[starts[e] : ends[e]] = res.results[e]["y"][: sizes[e]]
    return out
